# revision 29
# baseline (speedup 1.0000x reference)
"""Trainium2 Bass kernel for nn_Attention_16286515987100 (sparse_attention).

8 NeuronCores, data-parallel over B (one batch element per core).

Two NEFFs with a tiny host-side statistics reduction between them (the axon
bridge does not support on-device collective_compute; the exchanged data is
exactly what a single AllReduce would carry).

NEFF 1 (per core): LayerNorm -> projections -> normalized/centered feature
  tensors QT/KT (transposed via DMA-XBAR, no PE transposes) + fva, per-head
  gram matrices (one fused [128,128] matmul per tile+head), margin pass over
  a 256-key sample of the cosine scores in transposed layout, raw moment
  statistics shipped to the host.
host: reduce stats over the 8 cores, run the 3-layer weight-predictor MLP,
  compute global stds + temperature -> per-head scale tile sc.
NEFF 2 (per core): scale QT, head-pair-interleaved score matmul
  dots^T = a2*cos^T + b2*cov^T (K=128), exp (no max subtraction; |logit|<~3),
  O = P @ [f_v | 1] (softmax denominator rides along as the 65th column,
  PV matmuls software-pipelined one tile behind the scores), softmax
  denominators reciprocal'd in a [128,64] layout, per-head rescale, K=128
  head-paired output projection.
"""
import sys
import numpy as np

sys.path.insert(0, "/opt/trn_rl_repo")

import concourse.bass as bass
import concourse.bacc as bacc
import concourse.mybir as mybir
import concourse.tile as tile
from concourse.bass_utils import run_bass_kernel_spmd

F32 = mybir.dt.float32
BF16 = mybir.dt.bfloat16
AF = mybir.ActivationFunctionType
ALU = mybir.AluOpType
AX = mybir.AxisListType

N = 1024
DIM = 512
H = 8
D = 64
NT = N // 128
NCORES = 8
EPS = 1e-6
GAMMA = 0.01
MS = 128  # margin key-sample count
COV_SCALE = (0.001 / N) / (64.0 ** 0.5 + 1e-6)
M_TOT = float(H * 8 * N * N)

NSTAT = 104  # stats columns, see build_nc1


def build_nc1():
    nc = bacc.Bacc(None, target_bir_lowering=False, debug=False)

    q_e = nc.declare_dram_parameter("q", [N, DIM], F32, isOutput=False)
    k_e = nc.declare_dram_parameter("k", [N, DIM], F32, isOutput=False)
    v_e = nc.declare_dram_parameter("v", [N, DIM], F32, isOutput=False)
    Wf_e = nc.declare_dram_parameter("Wf", [DIM, DIM], BF16, isOutput=False)
    bW_e = nc.declare_dram_parameter("bW", [1, DIM], F32, isOutput=False)
    QT_o = nc.declare_dram_parameter("QTd", [128, H * N], BF16, isOutput=True)
    KT_o = nc.declare_dram_parameter("KTd", [128, H * N], BF16, isOutput=True)
    fva_o = nc.declare_dram_parameter("fvad", [128, H * NT * (D + 1)], BF16,
                                      isOutput=True)
    st_o = nc.declare_dram_parameter("stats", [128, NSTAT], F32, isOutput=True)
    qg_o = nc.declare_dram_parameter("qg", [2, DIM], F32, isOutput=True)

    with tile.TileContext(nc) as tc:
        with (
            tc.tile_pool(name="wpool", bufs=1) as wp,
            tc.tile_pool(name="persist", bufs=1) as pp,
        ):
            Wf_sb = wp.tile([128, 4, DIM], BF16, tag="Wf")
            for c in range(4):
                nc.gpsimd.dma_start(Wf_sb[:, c, :],
                                    Wf_e[c * 128:(c + 1) * 128, :])
            bW_row = wp.tile([1, DIM], F32, tag="rbW")
            nc.gpsimd.dma_start(bW_row[:], bW_e[:, :])
            bW_bc = wp.tile([128, DIM], F32, tag="bWb")
            nc.gpsimd.partition_broadcast(bW_bc[:], bW_row[:])
            ones_sb = wp.tile([128, 1], F32, tag="ones")
            nc.vector.memset(ones_sb[:], 1.0)
            ones_bf = wp.tile([128, 1], BF16, tag="onesb")
            nc.vector.memset(ones_bf[:], 1.0)
            c_gamma = wp.tile([128, 1], F32, tag="cgam")
            nc.vector.memset(c_gamma[:], GAMMA)
            c_neg1 = wp.tile([128, 1], F32, tag="cneg")
            nc.vector.memset(c_neg1[:], -1.0)
            c_eps = wp.tile([128, 1], F32, tag="ceps")
            nc.vector.memset(c_eps[:], 1e-5)

            QT = pp.tile([128, H, N], BF16, tag="QT")
            KT = pp.tile([128, H, N], BF16, tag="KT")
            fva = pp.tile([128, H, NT, D + 1], BF16, tag="fva")
            nc.gpsimd.memset(fva[:, :, :, D:D + 1], 1.0)
            facc_q = pp.tile([128, DIM], F32, tag="faq")
            facc_k = pp.tile([128, DIM], F32, tag="fak")
            nc.vector.memset(facc_q[:], 0.0)
            nc.vector.memset(facc_k[:], 0.0)
            Gq_sb = pp.tile([128, H, 128], F32, tag="Gq")
            Gk_sb = pp.tile([128, H, 128], F32, tag="Gk")
            sqv = pp.tile([64, H, 2], F32, tag="sqv")  # [:, h, 0]=q, 1=k
            sqv_bf = pp.tile([64, H, 2], BF16, tag="sqvb")
            stats = pp.tile([128, NSTAT], F32, tag="stm")
            nc.vector.memset(stats[:], 0.0)
            qg_sb = pp.tile([1, DIM], F32, tag="qgs")

            with (
                tc.tile_pool(name="stageA", bufs=3) as sp,
                tc.tile_pool(name="psA", bufs=2, space="PSUM") as psA,
            ):
                def prefetch(x_e):
                    xt8 = sp.tile([128, NT, DIM], F32, tag="xt8", bufs=2)
                    for nt in range(NT):
                        nc.sync.dma_start(xt8[:, nt, :],
                                          x_e[nt * 128:(nt + 1) * 128, :])
                    return xt8

                def ln_tile(xt8, nt):
                    """LN -> xn bf16, XBAR-transpose -> xnT,
                    project -> psum fp [128, DIM] f32."""
                    xt = xt8[:, nt, :]
                    bns = sp.tile([128, 6], F32, tag="bns")
                    nc.vector.bn_stats(bns[:], xt[:])
                    mv = sp.tile([128, 2], F32, tag="mv")
                    nc.vector.bn_aggr(mv[:], bns[:])
                    rstd = sp.tile([128, 1], F32, tag="lnrstd")
                    nc.scalar.activation(rstd[:], mv[:, 1:2], AF.Sqrt,
                                         bias=c_eps[:])
                    nc.vector.reciprocal(rstd[:], rstd[:])
                    nb = sp.tile([128, 1], F32, tag="lnnb")
                    nc.vector.scalar_tensor_tensor(
                        nb[:], mv[:, 0:1], -1.0, rstd[:], ALU.mult, ALU.mult)
                    xn = sp.tile([128, DIM], BF16, tag="xn")
                    nc.scalar.activation(xn[:], xt[:], AF.Identity, bias=nb[:],
                                         scale=rstd[:])
                    xnT = sp.tile([128, 4, 128], BF16, tag="xnT")
                    nc.sync.dma_start_transpose(xnT[:], xn[:])
                    fp = psA.tile([128, DIM], F32, tag="fproj")
                    for c in range(4):
                        nc.tensor.matmul(fp[:], xnT[:, c, :], Wf_sb[:, c, :],
                                         start=(c == 0), stop=(c == 3))
                    return fp

                # ---- K then Q (margin needs KT; K first) ----
                psG_cm = tc.tile_pool(name="psG", bufs=1, space="PSUM")
                psG = psG_cm.__enter__()
                g_ps = psG.tile([128, H, 128], F32, tag="g")
                xt8s = [prefetch(k_e), prefetch(q_e)]
                xt8v = [None]
                for ti, (x_e, T_sb, T_o, G_sb, facc, scol) in enumerate([
                        (k_e, KT, KT_o, Gk_sb, facc_k, 1),
                        (q_e, QT, QT_o, Gq_sb, facc_q, 0)]):
                    xt8 = xt8s[ti]
                    QQt8 = sp.tile([128, NT, H, 128], BF16, tag="QQt8", bufs=2)
                    for nt in range(NT):
                        fp = ln_tile(xt8, nt)
                        ft = sp.tile([128, DIM], F32, tag="ft")
                        nc.vector.scalar_tensor_tensor(
                            ft[:], fp[:], 1.0, bW_bc[:], ALU.mult, ALU.add)
                        nc.gpsimd.tensor_tensor(facc[:], facc[:], ft[:],
                                                ALU.add)
                        ftsq = sp.tile([128, DIM], BF16, tag="ftsq")
                        nc.scalar.activation(ftsq[:], ft[:], AF.Square)
                        n2 = sp.tile([128, H], F32, tag="n2")
                        nc.vector.tensor_reduce(
                            n2[:], ftsq[:].rearrange("p (h d) -> p h d", d=D),
                            axis=AX.X, op=ALU.add)
                        mudr = sp.tile([128, H], F32, tag="mudr")
                        nc.vector.tensor_reduce(
                            mudr[:], ft[:].rearrange("p (h d) -> p h d", d=D),
                            axis=AX.X, op=ALU.add)

                        rq = sp.tile([128, H], F32, tag="rq")
                        nc.scalar.activation(rq[:], n2[:], AF.Sqrt)
                        nc.vector.reciprocal(rq[:], rq[:])
                        QQt = QQt8[:, nt, :, :]
                        ftv = ft[:].rearrange("p (h d) -> p h d", d=D)
                        nc.gpsimd.tensor_tensor(
                            QQt[:, :, 0:D], ftv,
                            rq[:, :, None].broadcast_to([128, H, D]), ALU.mult)
                        nc.vector.scalar_tensor_tensor(
                            QQt[:, :, D:128],
                            mudr[:, :, None].broadcast_to([128, H, D]),
                            -1.0 / D, ftv, ALU.mult, ALU.add)
                        for h in range(H):
                            # start/stop mark per-PSUM-bank accumulation
                            # groups (heads 0-3 bank A, 4-7 bank B)
                            nc.tensor.matmul(g_ps[:, h, :], QQt[:, h, :],
                                             QQt[:, h, :],
                                             start=(nt == 0 and h in (0, 4)),
                                             stop=(nt == NT - 1
                                                   and h in (3, 7)))
                    if ti == 1:
                        xt8v[0] = prefetch(v_e)
                    for nt in range(NT):
                        nc.sync.dma_start_transpose(
                            T_sb[:, :, nt * 128:(nt + 1) * 128],
                            QQt8[:, nt, :, :].rearrange("p h c -> p (h c)"))
                    nc.vector.tensor_copy(G_sb[:], g_ps[:])
                    # per-head, per-feature sums of normalized features
                    # (split across Act/DVE so neither queue stalls the
                    # next pass's LayerNorm chain)
                    for h in range(H):
                        if h % 2 == 0:
                            jnk = sp.tile([64, N], F32, tag="jsv")
                            nc.scalar.activation(
                                jnk[:], T_sb[0:64, h, :], AF.Copy,
                                accum_out=sqv[:, h, scol:scol + 1])
                        else:
                            jnv = sp.tile([64, N], BF16, tag="jsvv")
                            nc.vector.tensor_scalar(
                                jnv[:], T_sb[0:64, h, :], 1.0, None,
                                ALU.mult, ALU.add,
                                accum_out=sqv[:, h, scol:scol + 1])
                    # persist transposed features to HBM for NEFF2
                    for h in range(H):
                        nc.gpsimd.dma_start(T_o[:, h * N:(h + 1) * N],
                                            T_sb[:, h, :])
                nc.vector.tensor_copy(sqv_bf[:], sqv[:])
                psG_cm.__exit__(None, None, None)

                # ---- V pass ----
                for nt in range(NT):
                    fp = ln_tile(xt8v[0], nt)
                    nc.vector.scalar_tensor_tensor(
                        fva[:, :, nt, 0:D],
                        fp[:].rearrange("p (h d) -> p h d", d=D), 1.0,
                        bW_bc[:].rearrange("p (h d) -> p h d", d=D),
                        ALU.mult, ALU.add)
                for h in range(H):
                    nc.gpsimd.dma_start(
                        fva_o[:, h * NT * (D + 1):(h + 1) * NT * (D + 1)],
                        fva[:, h, :, :])

                # ---- margin + rcs pass (transposed layout) ----
                with (
                    tc.tile_pool(name="p1", bufs=3) as sp1,
                    tc.tile_pool(name="psM", bufs=2, space="PSUM") as psM,
                    tc.tile_pool(name="psV", bufs=2, space="PSUM") as psV,
                ):
                    for h in range(H):
                        for g in range(2):
                            hg = h * 2 + g
                            qs = QT[0:64, h, g * 512:(g + 1) * 512]
                            relus = []
                            for ms in range(MS // 128):
                                csp = psM.tile([128, 512], F32, tag="csp")
                                nc.tensor.matmul(
                                    csp[:],
                                    KT[0:64, h, ms * 128:(ms + 1) * 128],
                                    qs, start=True, stop=True)
                                rl = sp1.tile([128, 512], BF16, tag="rl")
                                # accum_out rides along: per-key relu sums
                                # (host sums partitions -> m1)
                                nc.scalar.activation(
                                    rl[:], csp[:], AF.Relu, bias=c_gamma[:],
                                    scale=c_neg1[:],
                                    accum_out=stats[:, 40 + 2 * hg + ms:
                                                    41 + 2 * hg + ms])
                                relus.append(rl)
                            vr_ps = psV.tile([1, 512], F32, tag="vrp")
                            for ms, rl in enumerate(relus):
                                nc.tensor.matmul(
                                    vr_ps[:], ones_bf[:], rl[:],
                                    start=(ms == 0),
                                    stop=(ms == len(relus) - 1))
                            rcs_ps = psV.tile([1, 512], F32, tag="rcsp")
                            nc.tensor.matmul(
                                rcs_ps[:], sqv_bf[:, h, 1:2], qs,
                                start=True, stop=True)
                            jm2 = sp1.tile([1, 512], BF16, tag="jm2")
                            nc.scalar.activation(
                                jm2[:], vr_ps[:], AF.Square,
                                accum_out=stats[0:1, 72 + hg:73 + hg])
                            vr_sb = sp1.tile([1, 512], F32, tag="vrsb")
                            nc.vector.tensor_copy(vr_sb[:], vr_ps[:])
                            jm3 = sp1.tile([1, 512], F32, tag="jm3")
                            nc.vector.scalar_tensor_tensor(
                                jm3[:], vr_sb[:], 1.0, rcs_ps[:], ALU.mult,
                                ALU.mult,
                                accum_out=stats[0:1, 88 + hg:89 + hg])
                    # gram contractions -> stats cols 0..23
                    for h in range(H):
                        j1 = sp1.tile([64, 64], F32, tag="jg")
                        nc.vector.scalar_tensor_tensor(
                            j1[:], Gq_sb[0:64, h, 0:64], 1.0,
                            Gk_sb[0:64, h, 0:64], ALU.mult, ALU.mult,
                            accum_out=stats[0:64, h:h + 1])
                        j2 = sp1.tile([64, 64], F32, tag="jg")
                        nc.vector.scalar_tensor_tensor(
                            j2[:], Gq_sb[64:128, h, 64:128], 1.0,
                            Gk_sb[64:128, h, 64:128], ALU.mult, ALU.mult,
                            accum_out=stats[0:64, 8 + h:9 + h])
                        j3 = sp1.tile([64, 64], F32, tag="jg")
                        nc.vector.scalar_tensor_tensor(
                            j3[:], Gq_sb[0:64, h, 64:128], 1.0,
                            Gk_sb[0:64, h, 64:128], ALU.mult, ALU.mult,
                            accum_out=stats[0:64, 16 + h:17 + h])
                    # sqv/skv -> stats cols 24..39
                    nc.vector.tensor_copy(stats[0:64, 24:32], sqv[:, :, 0])
                    nc.vector.tensor_copy(stats[0:64, 32:40], sqv[:, :, 1])

                # ---- global feature sums (q_g/k_g) ----
                with tc.tile_pool(name="psQ", bufs=2, space="PSUM") as psQ:
                    qgq_ps = psQ.tile([1, DIM], F32, tag="qgp")
                    nc.tensor.matmul(qgq_ps[:], ones_sb[:], facc_q[:],
                                     start=True, stop=True)
                    nc.vector.tensor_copy(qg_sb[:], qgq_ps[:])
                    nc.sync.dma_start(qg_o[0:1, :], qg_sb[:])
                    qgk_ps = psQ.tile([1, DIM], F32, tag="qgp")
                    nc.tensor.matmul(qgk_ps[:], ones_sb[:], facc_k[:],
                                     start=True, stop=True)
                    kg_sb = sp.tile([1, DIM], F32, tag="kgs")
                    nc.vector.tensor_copy(kg_sb[:], qgk_ps[:])
                    nc.sync.dma_start(qg_o[1:2, :], kg_sb[:])
                nc.sync.dma_start(st_o[:, :], stats[:])

    nc.finalize()
    return nc


def build_nc2():
    nc = bacc.Bacc(None, target_bir_lowering=False, debug=False)

    QT_e = nc.declare_dram_parameter("QTd", [128, H * N], BF16, isOutput=False)
    KT_e = nc.declare_dram_parameter("KTd", [128, H * N], BF16, isOutput=False)
    fva_e = nc.declare_dram_parameter("fvad", [128, H * NT * (D + 1)], BF16,
                                      isOutput=False)
    sc_e = nc.declare_dram_parameter("sc", [128, H], F32, isOutput=False)
    Wo_e = nc.declare_dram_parameter("W_out", [DIM, DIM], BF16, isOutput=False)
    bo_e = nc.declare_dram_parameter("b_out", [1, DIM], F32, isOutput=False)
    out_e = nc.declare_dram_parameter("out", [N, DIM], F32, isOutput=True)

    with tile.TileContext(nc) as tc:
        with (
            tc.tile_pool(name="wpool2", bufs=1) as wp,
            tc.tile_pool(name="persist2", bufs=1) as pp,
        ):
            Wo_sb = wp.tile([128, 4, DIM], BF16, tag="Wo")
            for hp in range(4):
                nc.gpsimd.dma_start(Wo_sb[:, hp, :],
                                    Wo_e[hp * 128:(hp + 1) * 128, :])
            bo_row = wp.tile([1, DIM], F32, tag="rbo")
            nc.gpsimd.dma_start(bo_row[:], bo_e[:, :])
            bo_bc = wp.tile([128, DIM], F32, tag="bob")
            nc.gpsimd.partition_broadcast(bo_bc[:], bo_row[:])
            sc_sb = wp.tile([128, H], F32, tag="sc")
            nc.gpsimd.dma_start(sc_sb[:], sc_e[:, :])

            QT = pp.tile([128, H, N], BF16, tag="QT")
            KT = pp.tile([128, H, N], BF16, tag="KT")
            fva = pp.tile([128, H, NT, D + 1], BF16, tag="fva")
            OT2 = pp.tile([128, 4, N], BF16, tag="OT2")
            den = pp.tile([128, D], F32, tag="den")
            rec = pp.tile([128, D], F32, tag="rec")
            rw8 = pp.tile([1, H, N], F32, tag="rw8")
            for h in range(H):
                nc.sync.dma_start(QT[:, h, :], QT_e[:, h * N:(h + 1) * N])
                nc.scalar.dma_start(KT[:, h, :], KT_e[:, h * N:(h + 1) * N])
                nc.gpsimd.dma_start(
                    fva[:, h, :, :],
                    fva_e[:, h * NT * (D + 1):(h + 1) * NT * (D + 1)])
            for h in range(H):
                nc.vector.tensor_scalar(QT[:, h, :], QT[:, h, :],
                                        sc_sb[:, h:h + 1], None, ALU.mult)

            with (
                tc.tile_pool(name="p2", bufs=4) as sp2,
                tc.tile_pool(name="psS", bufs=2, space="PSUM") as psS,
                tc.tile_pool(name="psO", bufs=2, space="PSUM") as psO,
            ):
                for i in range(4):
                    ops = [psO.tile([D + 1, N], F32, tag="ops",
                                    name=f"ops{i}_{jj}")
                           for jj in range(2)]
                    ets = [None, None]
                    for mt in range(NT):
                        net = [None, None]
                        for j in range(2):
                            h = 2 * i + j
                            sps = psS.tile([128, N], F32, tag="sps")
                            nc.tensor.matmul(
                                sps[:, 0:512],
                                KT[:, h, mt * 128:(mt + 1) * 128],
                                QT[:, h, 0:512], start=True, stop=True)
                            nc.tensor.matmul(
                                sps[:, 512:1024],
                                KT[:, h, mt * 128:(mt + 1) * 128],
                                QT[:, h, 512:1024], start=True, stop=True)
                            et = sp2.tile([128, N], BF16, tag="et")
                            nc.scalar.activation(et[:], sps[:], AF.Exp)
                            net[j] = et
                        # PV one tile behind the scores hides the exp latency
                        if mt > 0:
                            for j in range(2):
                                h = 2 * i + j
                                nc.tensor.matmul(
                                    ops[j][:, 0:512], fva[:, h, mt - 1, :],
                                    ets[j][:, 0:512], start=(mt == 1),
                                    stop=False)
                                nc.tensor.matmul(
                                    ops[j][:, 512:1024], fva[:, h, mt - 1, :],
                                    ets[j][:, 512:1024], start=(mt == 1),
                                    stop=False)
                        ets = net
                    for j in range(2):
                        h = 2 * i + j
                        nc.tensor.matmul(ops[j][:, 0:512],
                                         fva[:, h, NT - 1, :],
                                         ets[j][:, 0:512], start=False,
                                         stop=True)
                        nc.tensor.matmul(ops[j][:, 512:1024],
                                         fva[:, h, NT - 1, :],
                                         ets[j][:, 512:1024], start=False,
                                         stop=True)
                    # epilogue: evict PSUM fast, then normalize from SBUF
                    otr = [None, None]
                    for j in range(2):
                        o = sp2.tile([D + 1, N], F32, tag="otr")
                        nc.vector.tensor_copy(o[:], ops[j][:])
                        otr[j] = o
                    for j in range(2):
                        h = 2 * i + j
                        nc.sync.dma_start(
                            den[h * 16:(h + 1) * 16, :],
                            otr[j][D:D + 1, :].rearrange(
                                "o (c f) -> o c f", f=D))
                    nc.vector.reciprocal(rec[2 * i * 16:(2 * i + 2) * 16, :],
                                         den[2 * i * 16:(2 * i + 2) * 16, :])
                    for j in range(2):
                        h = 2 * i + j
                        nc.sync.dma_start(
                            rw8[0:1, h, :].rearrange("o (c f) -> o c f", f=D),
                            rec[h * 16:(h + 1) * 16, :])
                        rw = sp2.tile([64, N], F32, tag="rw")
                        nc.gpsimd.partition_broadcast(rw[:], rw8[0:1, h, :],
                                                      channels=64)
                        if j == 0:
                            nc.vector.tensor_tensor(OT2[0:64, i, :],
                                                    otr[j][0:D, :], rw[:],
                                                    ALU.mult)
                        else:
                            ott = sp2.tile([64, N], BF16, tag="ott")
                            nc.vector.tensor_tensor(ott[:], otr[j][0:D, :],
                                                    rw[:], ALU.mult)
                            nc.sync.dma_start(OT2[64:128, i, :], ott[:])

            with (
                tc.tile_pool(name="p3", bufs=3) as sp3,
                tc.tile_pool(name="psF", bufs=2, space="PSUM") as psF,
            ):
                for nt in range(NT):
                    fps = psF.tile([128, DIM], F32, tag="fps")
                    for i in range(4):
                        nc.tensor.matmul(fps[:],
                                         OT2[:, i, nt * 128:(nt + 1) * 128],
                                         Wo_sb[:, i, :], start=(i == 0),
                                         stop=(i == 3))
                    obt = sp3.tile([128, DIM], F32, tag="obt")
                    nc.vector.tensor_tensor(obt[:], fps[:], bo_bc[:], ALU.add)
                    nc.sync.dma_start(out_e[nt * 128:(nt + 1) * 128, :],
                                      obt[:])

    nc.finalize()
    return nc


_NC1 = None
_NC2 = None


def _get_ncs():
    global _NC1, _NC2
    if _NC1 is None:
        _NC1 = build_nc1()
        _NC2 = build_nc2()
    return _NC1, _NC2


def host_mid(stats_list, qg_list, wp_W1, wp_b1, wp_ln_g, wp_ln_b, wp_W2,
             wp_b2, wp_W3, wp_b3, weight_temp):
    """Reduce per-core stats, run the weight-predictor MLP + global stds,
    return the [128, H] pass-2 scale tile (a2 rows 0:64, b2 rows 64:128)."""
    f8 = np.float64
    # --- global feature means ---
    qg = np.zeros((2, DIM), f8)
    for g in qg_list:
        qg += np.asarray(g, f8)
    q_g = qg[0].reshape(H, D) / (NCORES * N)
    k_g = qg[1].reshape(H, D) / (NCORES * N)

    # --- per-head moment sums (same conventions the baseline validated) ---
    S2C = np.zeros(H, f8)
    S2V = np.zeros(H, f8)
    SCV = np.zeros(H, f8)
    S1C = np.zeros(H, f8)
    S1VAR = np.zeros(H, f8)
    S2VAR = np.zeros(H, f8)
    SCVAR = np.zeros(H, f8)
    for s in stats_list:
        s = np.asarray(s, f8)
        S2C += s[0:64, 0:8].sum(axis=0)
        S2V += s[0:64, 8:16].sum(axis=0)
        SCV += s[0:64, 16:24].sum(axis=0)
        sqv = s[0:64, 24:32]
        skv = s[0:64, 32:40]
        S1C += (sqv * skv).sum(axis=0)
        # m1: per-(hg, ms) relu accum columns, summed over key partitions
        m1 = s[:, 40:72].sum(axis=0).reshape(H, 4).sum(axis=1)
        m2 = s[0, 72:88].reshape(H, 2).sum(axis=1)
        m3 = s[0, 88:104].reshape(H, 2).sum(axis=1)
        S1VAR += m1 / MS
        S2VAR += m2 / (MS * MS)
        SCVAR += m3 / MS
    S2V *= COV_SCALE * COV_SCALE
    SCV *= COV_SCALE

    # --- weight predictor MLP ---
    z = np.concatenate([q_g, k_g], axis=-1) @ np.asarray(wp_W1, f8) \
        + np.asarray(wp_b1, f8)
    mu = z.mean(-1, keepdims=True)
    var = z.var(-1, keepdims=True)
    z = (z - mu) / np.sqrt(var + 1e-5) * np.asarray(wp_ln_g, f8) \
        + np.asarray(wp_ln_b, f8)
    z = np.maximum(z, 0)
    z = np.maximum(z @ np.asarray(wp_W2, f8) + np.asarray(wp_b2, f8), 0)
    logits = z @ np.asarray(wp_W3, f8) + np.asarray(wp_b3, f8)
    e = np.exp(logits - logits.max(-1, keepdims=True))
    p = e / e.sum(-1, keepdims=True)
    wt = np.clip(np.asarray(weight_temp, f8), 0.1, 10.0)
    e2 = np.exp(p / wt - (p / wt).max(-1, keepdims=True))
    w = e2 / e2.sum(-1, keepdims=True)
    w = w * 0.7 + 0.1
    cw, covw, vw = w[:, 0], w[:, 1], w[:, 2]

    def std1(s1, s2):
        return np.sqrt(max((s2 - s1 * s1 / M_TOT) / (M_TOT - 1.0), 0.0))

    cos_n = std1(S1C.sum(), S2C.sum()) + EPS
    cov_n = std1(0.0, S2V.sum()) + EPS
    var_n = std1(S1VAR.sum(), S2VAR.sum()) + EPS
    A = cw / cos_n
    Bc = covw / cov_n * 0.3
    C = vw / var_n * 0.3
    S1d = (A * S1C + C * S1VAR).sum()
    S2d = (A * A * S2C + Bc * Bc * S2V + C * C * S2VAR + 2 * A * Bc * SCV
           + 2 * A * C * SCVAR).sum()
    temp = np.clip(0.5 + std1(S1d, S2d), 0.3, 3.0)
    a2 = A / temp
    b2 = Bc / temp * COV_SCALE
    sc = np.zeros((128, H), np.float32)
    sc[0:64, :] = a2[None, :]
    sc[64:128, :] = b2[None, :]
    return sc


def make_in_maps1(q, k, v, ln_g, ln_b, W_in):
    import ml_dtypes
    f = np.float32
    bf = ml_dtypes.bfloat16
    Wf = (np.asarray(ln_g, f)[:, None] * np.asarray(W_in, f)).astype(bf)
    bW = (np.asarray(ln_b, f) @ np.asarray(W_in, f))[None, :].astype(f)
    shared = dict(Wf=Wf, bW=bW)
    maps = []
    for b in range(NCORES):
        m = dict(shared)
        m["q"] = np.ascontiguousarray(np.asarray(q, f)[b])
        m["k"] = np.ascontiguousarray(np.asarray(k, f)[b])
        m["v"] = np.ascontiguousarray(np.asarray(v, f)[b])
        maps.append(m)
    return maps


def make_in_maps2(r1, sc, W_out, b_out):
    import ml_dtypes
    f = np.float32
    Wo = np.asarray(W_out, f).astype(ml_dtypes.bfloat16)
    bo = np.asarray(b_out, f)[None, :]
    maps2 = []
    for b in range(NCORES):
        maps2.append(dict(QTd=np.asarray(r1[b]["QTd"]),
                          KTd=np.asarray(r1[b]["KTd"]),
                          fvad=np.asarray(r1[b]["fvad"]),
                          sc=sc, W_out=Wo, b_out=bo))
    return maps2


def kernel(**inputs) -> np.ndarray:
    nc1, nc2 = _get_ncs()
    maps1 = make_in_maps1(inputs["q"], inputs["k"], inputs["v"],
                          inputs["ln_g"], inputs["ln_b"], inputs["W_in"])
    res1 = run_bass_kernel_spmd(nc1, maps1, core_ids=list(range(NCORES)))
    r1 = res1.results
    sc = host_mid([r1[b]["stats"] for b in range(NCORES)],
                  [r1[b]["qg"] for b in range(NCORES)],
                  inputs["wp_W1"], inputs["wp_b1"], inputs["wp_ln_g"],
                  inputs["wp_ln_b"], inputs["wp_W2"], inputs["wp_b2"],
                  inputs["wp_W3"], inputs["wp_b3"], inputs["weight_temp"])
    maps2 = make_in_maps2(r1, sc, inputs["W_out"], inputs["b_out"])
    res2 = run_bass_kernel_spmd(nc2, maps2, core_ids=list(range(NCORES)))
    r2 = res2.results
    return np.stack([np.asarray(r2[b]["out"]) for b in range(NCORES)], axis=0)


if __name__ == "__main__":
    _get_ncs()
    print("built ok")


# revision 30
# speedup vs baseline: 1.0137x; 1.0137x over previous
"""Trainium2 Bass kernel for nn_Attention_16286515987100 (sparse_attention).

8 NeuronCores, data-parallel over B (one batch element per core).

Two NEFFs with a tiny host-side statistics reduction between them (the axon
bridge does not support on-device collective_compute; the exchanged data is
exactly what a single AllReduce would carry).

NEFF 1 (per core): LayerNorm -> projections -> normalized/centered feature
  tensors QT/KT (transposed via DMA-XBAR, no PE transposes) + fva, per-head
  gram matrices (one fused [128,128] matmul per tile+head), margin pass over
  a 256-key sample of the cosine scores in transposed layout, raw moment
  statistics shipped to the host.
host: reduce stats over the 8 cores, run the 3-layer weight-predictor MLP,
  compute global stds + temperature -> per-head scale tile sc.
NEFF 2 (per core): scale QT, head-pair-interleaved score matmul
  dots^T = a2*cos^T + b2*cov^T (K=128), exp (no max subtraction; |logit|<~3),
  O = P @ [f_v | 1] (softmax denominator rides along as the 65th column,
  PV matmuls software-pipelined one tile behind the scores), softmax
  denominators reciprocal'd in a [128,64] layout, per-head rescale, K=128
  head-paired output projection.
"""
import sys
import numpy as np

sys.path.insert(0, "/opt/trn_rl_repo")

import concourse.bass as bass
import concourse.bacc as bacc
import concourse.mybir as mybir
import concourse.tile as tile
from concourse.bass_utils import run_bass_kernel_spmd

F32 = mybir.dt.float32
BF16 = mybir.dt.bfloat16
AF = mybir.ActivationFunctionType
ALU = mybir.AluOpType
AX = mybir.AxisListType

N = 1024
DIM = 512
H = 8
D = 64
NT = N // 128
NCORES = 8
EPS = 1e-6
GAMMA = 0.01
MS = 128  # margin key-sample count
COV_SCALE = (0.001 / N) / (64.0 ** 0.5 + 1e-6)
M_TOT = float(H * 8 * N * N)

NSTAT = 104  # stats columns, see build_nc1


def build_nc1():
    nc = bacc.Bacc(None, target_bir_lowering=False, debug=False)

    q_e = nc.declare_dram_parameter("q", [N, DIM], F32, isOutput=False)
    k_e = nc.declare_dram_parameter("k", [N, DIM], F32, isOutput=False)
    v_e = nc.declare_dram_parameter("v", [N, DIM], F32, isOutput=False)
    Wf_e = nc.declare_dram_parameter("Wf", [DIM, DIM], BF16, isOutput=False)
    bW_e = nc.declare_dram_parameter("bW", [1, DIM], F32, isOutput=False)
    QT_o = nc.declare_dram_parameter("QTd", [128, H * N], BF16, isOutput=True)
    KT_o = nc.declare_dram_parameter("KTd", [128, H * N], BF16, isOutput=True)
    fva_o = nc.declare_dram_parameter("fvad", [128, H * NT * (D + 1)], BF16,
                                      isOutput=True)
    st_o = nc.declare_dram_parameter("stats", [128, NSTAT], F32, isOutput=True)
    qg_o = nc.declare_dram_parameter("qg", [2, DIM], F32, isOutput=True)

    with tile.TileContext(nc) as tc:
        with (
            tc.tile_pool(name="wpool", bufs=1) as wp,
            tc.tile_pool(name="persist", bufs=1) as pp,
        ):
            Wf_sb = wp.tile([128, 4, DIM], BF16, tag="Wf")
            for c in range(4):
                nc.gpsimd.dma_start(Wf_sb[:, c, :],
                                    Wf_e[c * 128:(c + 1) * 128, :])
            bW_row = wp.tile([1, DIM], F32, tag="rbW")
            nc.gpsimd.dma_start(bW_row[:], bW_e[:, :])
            bW_bc = wp.tile([128, DIM], F32, tag="bWb")
            nc.gpsimd.partition_broadcast(bW_bc[:], bW_row[:])
            ones_sb = wp.tile([128, 1], F32, tag="ones")
            nc.vector.memset(ones_sb[:], 1.0)
            ones_bf = wp.tile([128, 1], BF16, tag="onesb")
            nc.vector.memset(ones_bf[:], 1.0)
            c_gamma = wp.tile([128, 1], F32, tag="cgam")
            nc.vector.memset(c_gamma[:], GAMMA)
            c_neg1 = wp.tile([128, 1], F32, tag="cneg")
            nc.vector.memset(c_neg1[:], -1.0)
            c_eps = wp.tile([128, 1], F32, tag="ceps")
            nc.vector.memset(c_eps[:], 1e-5)

            QT = pp.tile([128, H, N], BF16, tag="QT")
            KT = pp.tile([128, H, N], BF16, tag="KT")
            fva = pp.tile([128, H, NT, D + 1], BF16, tag="fva")
            nc.gpsimd.memset(fva[:, :, :, D:D + 1], 1.0)
            facc_q = pp.tile([128, DIM], F32, tag="faq")
            facc_k = pp.tile([128, DIM], F32, tag="fak")
            nc.vector.memset(facc_q[:], 0.0)
            nc.vector.memset(facc_k[:], 0.0)
            Gq_sb = pp.tile([128, H, 128], F32, tag="Gq")
            Gk_sb = pp.tile([128, H, 128], F32, tag="Gk")
            sqv = pp.tile([64, H, 2], F32, tag="sqv")  # [:, h, 0]=q, 1=k
            sqv_bf = pp.tile([64, H, 2], BF16, tag="sqvb")
            stats = pp.tile([128, NSTAT], F32, tag="stm")
            nc.vector.memset(stats[:], 0.0)
            qg_sb = pp.tile([1, DIM], F32, tag="qgs")

            with (
                tc.tile_pool(name="stageA", bufs=3) as sp,
                tc.tile_pool(name="psA", bufs=2, space="PSUM") as psA,
            ):
                def prefetch(x_e):
                    xt8 = sp.tile([128, NT, DIM], F32, tag="xt8", bufs=2)
                    for nt in range(NT):
                        nc.sync.dma_start(xt8[:, nt, :],
                                          x_e[nt * 128:(nt + 1) * 128, :])
                    return xt8

                def ln_tile(xt8, nt):
                    """LN -> xn bf16, XBAR-transpose -> xnT,
                    project -> psum fp [128, DIM] f32."""
                    xt = xt8[:, nt, :]
                    bns = sp.tile([128, 6], F32, tag="bns")
                    nc.vector.bn_stats(bns[:], xt[:])
                    mv = sp.tile([128, 2], F32, tag="mv")
                    nc.vector.bn_aggr(mv[:], bns[:])
                    rstd = sp.tile([128, 1], F32, tag="lnrstd")
                    nc.scalar.activation(rstd[:], mv[:, 1:2], AF.Sqrt,
                                         bias=c_eps[:])
                    nc.vector.reciprocal(rstd[:], rstd[:])
                    nb = sp.tile([128, 1], F32, tag="lnnb")
                    nc.vector.scalar_tensor_tensor(
                        nb[:], mv[:, 0:1], -1.0, rstd[:], ALU.mult, ALU.mult)
                    xn = sp.tile([128, DIM], BF16, tag="xn")
                    nc.scalar.activation(xn[:], xt[:], AF.Identity, bias=nb[:],
                                         scale=rstd[:])
                    xnT = sp.tile([128, 4, 128], BF16, tag="xnT")
                    nc.sync.dma_start_transpose(xnT[:], xn[:])
                    fp = psA.tile([128, DIM], F32, tag="fproj")
                    for c in range(4):
                        nc.tensor.matmul(fp[:], xnT[:, c, :], Wf_sb[:, c, :],
                                         start=(c == 0), stop=(c == 3))
                    return fp

                # ---- K then Q (margin needs KT; K first) ----
                psG_cm = tc.tile_pool(name="psG", bufs=1, space="PSUM")
                psG = psG_cm.__enter__()
                g_ps = psG.tile([128, H, 128], F32, tag="g")
                xt8s = [prefetch(k_e), prefetch(q_e)]
                xt8v = [None]
                for ti, (x_e, T_sb, T_o, G_sb, facc, scol) in enumerate([
                        (k_e, KT, KT_o, Gk_sb, facc_k, 1),
                        (q_e, QT, QT_o, Gq_sb, facc_q, 0)]):
                    xt8 = xt8s[ti]
                    QQt8 = sp.tile([128, NT, H, 128], BF16, tag="QQt8", bufs=2)
                    for nt in range(NT):
                        fp = ln_tile(xt8, nt)
                        ft = sp.tile([128, DIM], F32, tag="ft")
                        nc.vector.scalar_tensor_tensor(
                            ft[:], fp[:], 1.0, bW_bc[:], ALU.mult, ALU.add)
                        nc.gpsimd.tensor_tensor(facc[:], facc[:], ft[:],
                                                ALU.add)
                        ftsq = sp.tile([128, DIM], BF16, tag="ftsq")
                        nc.scalar.activation(ftsq[:], ft[:], AF.Square)
                        n2 = sp.tile([128, H], F32, tag="n2")
                        nc.vector.tensor_reduce(
                            n2[:], ftsq[:].rearrange("p (h d) -> p h d", d=D),
                            axis=AX.X, op=ALU.add)
                        mudr = sp.tile([128, H], F32, tag="mudr")
                        nc.vector.tensor_reduce(
                            mudr[:], ft[:].rearrange("p (h d) -> p h d", d=D),
                            axis=AX.X, op=ALU.add)

                        rq = sp.tile([128, H], F32, tag="rq")
                        nc.scalar.activation(rq[:], n2[:], AF.Sqrt)
                        nc.vector.reciprocal(rq[:], rq[:])
                        QQt = QQt8[:, nt, :, :]
                        ftv = ft[:].rearrange("p (h d) -> p h d", d=D)
                        nc.gpsimd.tensor_tensor(
                            QQt[:, :, 0:D], ftv,
                            rq[:, :, None].broadcast_to([128, H, D]), ALU.mult)
                        nc.vector.scalar_tensor_tensor(
                            QQt[:, :, D:128],
                            mudr[:, :, None].broadcast_to([128, H, D]),
                            -1.0 / D, ftv, ALU.mult, ALU.add)
                        for h in range(H):
                            # start/stop mark per-PSUM-bank accumulation
                            # groups (heads 0-3 bank A, 4-7 bank B)
                            nc.tensor.matmul(g_ps[:, h, :], QQt[:, h, :],
                                             QQt[:, h, :],
                                             start=(nt == 0 and h in (0, 4)),
                                             stop=(nt == NT - 1
                                                   and h in (3, 7)))
                    if ti == 1:
                        xt8v[0] = prefetch(v_e)
                    for nt in range(NT):
                        nc.sync.dma_start_transpose(
                            T_sb[:, :, nt * 128:(nt + 1) * 128],
                            QQt8[:, nt, :, :].rearrange("p h c -> p (h c)"))
                    nc.vector.tensor_copy(G_sb[:], g_ps[:])
                    # per-head, per-feature sums of normalized features
                    # (split across Act/DVE so neither queue stalls the
                    # next pass's LayerNorm chain)
                    for h in range(H):
                        if h % 2 == 0:
                            jnk = sp.tile([64, N], F32, tag="jsv")
                            nc.scalar.activation(
                                jnk[:], T_sb[0:64, h, :], AF.Copy,
                                accum_out=sqv[:, h, scol:scol + 1])
                        else:
                            jnv = sp.tile([64, N], BF16, tag="jsvv")
                            nc.vector.tensor_scalar(
                                jnv[:], T_sb[0:64, h, :], 1.0, None,
                                ALU.mult, ALU.add,
                                accum_out=sqv[:, h, scol:scol + 1])
                    # persist transposed features to HBM for NEFF2
                    for h in range(H):
                        nc.gpsimd.dma_start(T_o[:, h * N:(h + 1) * N],
                                            T_sb[:, h, :])
                nc.vector.tensor_copy(sqv_bf[:], sqv[:])
                psG_cm.__exit__(None, None, None)

                # ---- V pass ----
                for nt in range(NT):
                    fp = ln_tile(xt8v[0], nt)
                    nc.vector.scalar_tensor_tensor(
                        fva[:, :, nt, 0:D],
                        fp[:].rearrange("p (h d) -> p h d", d=D), 1.0,
                        bW_bc[:].rearrange("p (h d) -> p h d", d=D),
                        ALU.mult, ALU.add)
                for h in range(H):
                    nc.gpsimd.dma_start(
                        fva_o[:, h * NT * (D + 1):(h + 1) * NT * (D + 1)],
                        fva[:, h, :, :])

                # ---- margin + rcs pass (transposed layout) ----
                with (
                    tc.tile_pool(name="p1", bufs=3) as sp1,
                    tc.tile_pool(name="psM", bufs=2, space="PSUM") as psM,
                    tc.tile_pool(name="psV", bufs=2, space="PSUM") as psV,
                ):
                    for h in range(H):
                        for g in range(2):
                            hg = h * 2 + g
                            qs = QT[0:64, h, g * 512:(g + 1) * 512]
                            relus = []
                            for ms in range(MS // 128):
                                csp = psM.tile([128, 512], F32, tag="csp")
                                nc.tensor.matmul(
                                    csp[:],
                                    KT[0:64, h, ms * 128:(ms + 1) * 128],
                                    qs, start=True, stop=True)
                                rl = sp1.tile([128, 512], BF16, tag="rl")
                                # accum_out rides along: per-key relu sums
                                # (host sums partitions -> m1)
                                nc.scalar.activation(
                                    rl[:], csp[:], AF.Relu, bias=c_gamma[:],
                                    scale=c_neg1[:],
                                    accum_out=stats[:, 40 + 2 * hg + ms:
                                                    41 + 2 * hg + ms])
                                relus.append(rl)
                            vr_ps = psV.tile([1, 512], F32, tag="vrp")
                            for ms, rl in enumerate(relus):
                                nc.tensor.matmul(
                                    vr_ps[:], ones_bf[:], rl[:],
                                    start=(ms == 0),
                                    stop=(ms == len(relus) - 1))
                            rcs_ps = psV.tile([1, 512], F32, tag="rcsp")
                            nc.tensor.matmul(
                                rcs_ps[:], sqv_bf[:, h, 1:2], qs,
                                start=True, stop=True)
                            jm2 = sp1.tile([1, 512], BF16, tag="jm2")
                            nc.scalar.activation(
                                jm2[:], vr_ps[:], AF.Square,
                                accum_out=stats[0:1, 72 + hg:73 + hg])
                            vr_sb = sp1.tile([1, 512], F32, tag="vrsb")
                            nc.vector.tensor_copy(vr_sb[:], vr_ps[:])
                            jm3 = sp1.tile([1, 512], F32, tag="jm3")
                            nc.vector.scalar_tensor_tensor(
                                jm3[:], vr_sb[:], 1.0, rcs_ps[:], ALU.mult,
                                ALU.mult,
                                accum_out=stats[0:1, 88 + hg:89 + hg])
                    # gram contractions -> stats cols 0..23
                    for h in range(H):
                        j1 = sp1.tile([64, 64], F32, tag="jg")
                        nc.vector.scalar_tensor_tensor(
                            j1[:], Gq_sb[0:64, h, 0:64], 1.0,
                            Gk_sb[0:64, h, 0:64], ALU.mult, ALU.mult,
                            accum_out=stats[0:64, h:h + 1])
                        j2 = sp1.tile([64, 64], F32, tag="jg")
                        nc.vector.scalar_tensor_tensor(
                            j2[:], Gq_sb[64:128, h, 64:128], 1.0,
                            Gk_sb[64:128, h, 64:128], ALU.mult, ALU.mult,
                            accum_out=stats[0:64, 8 + h:9 + h])
                        j3 = sp1.tile([64, 64], F32, tag="jg")
                        nc.vector.scalar_tensor_tensor(
                            j3[:], Gq_sb[0:64, h, 64:128], 1.0,
                            Gk_sb[0:64, h, 64:128], ALU.mult, ALU.mult,
                            accum_out=stats[0:64, 16 + h:17 + h])
                    # sqv/skv -> stats cols 24..39
                    nc.vector.tensor_copy(stats[0:64, 24:32], sqv[:, :, 0])
                    nc.vector.tensor_copy(stats[0:64, 32:40], sqv[:, :, 1])

                # ---- global feature sums (q_g/k_g) ----
                with tc.tile_pool(name="psQ", bufs=2, space="PSUM") as psQ:
                    qgq_ps = psQ.tile([1, DIM], F32, tag="qgp")
                    nc.tensor.matmul(qgq_ps[:], ones_sb[:], facc_q[:],
                                     start=True, stop=True)
                    nc.vector.tensor_copy(qg_sb[:], qgq_ps[:])
                    nc.sync.dma_start(qg_o[0:1, :], qg_sb[:])
                    qgk_ps = psQ.tile([1, DIM], F32, tag="qgp")
                    nc.tensor.matmul(qgk_ps[:], ones_sb[:], facc_k[:],
                                     start=True, stop=True)
                    kg_sb = sp.tile([1, DIM], F32, tag="kgs")
                    nc.vector.tensor_copy(kg_sb[:], qgk_ps[:])
                    nc.sync.dma_start(qg_o[1:2, :], kg_sb[:])
                nc.sync.dma_start(st_o[:, :], stats[:])

    nc.finalize()
    return nc


def build_nc2():
    nc = bacc.Bacc(None, target_bir_lowering=False, debug=False)

    QT_e = nc.declare_dram_parameter("QTd", [128, H * N], BF16, isOutput=False)
    KT_e = nc.declare_dram_parameter("KTd", [128, H * N], BF16, isOutput=False)
    fva_e = nc.declare_dram_parameter("fvad", [128, H * NT * (D + 1)], BF16,
                                      isOutput=False)
    sc_e = nc.declare_dram_parameter("sc", [128, H], F32, isOutput=False)
    Wo_e = nc.declare_dram_parameter("W_out", [DIM, DIM], BF16, isOutput=False)
    bo_e = nc.declare_dram_parameter("b_out", [1, DIM], F32, isOutput=False)
    out_e = nc.declare_dram_parameter("out", [N, DIM], F32, isOutput=True)

    with tile.TileContext(nc) as tc:
        with (
            tc.tile_pool(name="wpool2", bufs=1) as wp,
            tc.tile_pool(name="persist2", bufs=1) as pp,
        ):
            Wo_sb = wp.tile([128, 4, DIM], BF16, tag="Wo")
            for hp in range(4):
                nc.gpsimd.dma_start(Wo_sb[:, hp, :],
                                    Wo_e[hp * 128:(hp + 1) * 128, :])
            bo_row = wp.tile([1, DIM], F32, tag="rbo")
            nc.gpsimd.dma_start(bo_row[:], bo_e[:, :])
            bo_bc = wp.tile([128, DIM], F32, tag="bob")
            nc.gpsimd.partition_broadcast(bo_bc[:], bo_row[:])
            sc_sb = wp.tile([128, H], F32, tag="sc")
            nc.gpsimd.dma_start(sc_sb[:], sc_e[:, :])

            QT = pp.tile([128, H, N], BF16, tag="QT")
            KT = pp.tile([128, H, N], BF16, tag="KT")
            fva = pp.tile([128, H, NT, D + 1], BF16, tag="fva")
            OT2 = pp.tile([128, 4, N], BF16, tag="OT2")
            den = pp.tile([128, D], F32, tag="den")
            rec = pp.tile([128, D], F32, tag="rec")
            rw8 = pp.tile([1, H, N], F32, tag="rw8")
            for h in range(H):
                nc.sync.dma_start(QT[:, h, :], QT_e[:, h * N:(h + 1) * N])
                nc.sync.dma_start(KT[:, h, :], KT_e[:, h * N:(h + 1) * N])
                nc.gpsimd.dma_start(
                    fva[:, h, :, :],
                    fva_e[:, h * NT * (D + 1):(h + 1) * NT * (D + 1)])
            for h in range(H):
                nc.vector.tensor_scalar(QT[:, h, :], QT[:, h, :],
                                        sc_sb[:, h:h + 1], None, ALU.mult)

            with (
                tc.tile_pool(name="p2", bufs=4) as sp2,
                tc.tile_pool(name="psS", bufs=2, space="PSUM") as psS,
                tc.tile_pool(name="psO", bufs=2, space="PSUM") as psO,
            ):
                for i in range(4):
                    ops = [psO.tile([D + 1, N], F32, tag="ops",
                                    name=f"ops{i}_{jj}")
                           for jj in range(2)]
                    ets = [None, None]
                    for mt in range(NT):
                        net = [None, None]
                        for j in range(2):
                            h = 2 * i + j
                            sps = psS.tile([128, N], F32, tag="sps")
                            nc.tensor.matmul(
                                sps[:, 0:512],
                                KT[:, h, mt * 128:(mt + 1) * 128],
                                QT[:, h, 0:512], start=True, stop=True)
                            nc.tensor.matmul(
                                sps[:, 512:1024],
                                KT[:, h, mt * 128:(mt + 1) * 128],
                                QT[:, h, 512:1024], start=True, stop=True)
                            et = sp2.tile([128, N], BF16, tag="et")
                            nc.scalar.activation(et[:], sps[:], AF.Exp)
                            net[j] = et
                        # PV one tile behind the scores hides the exp latency
                        if mt > 0:
                            for j in range(2):
                                h = 2 * i + j
                                nc.tensor.matmul(
                                    ops[j][:, 0:512], fva[:, h, mt - 1, :],
                                    ets[j][:, 0:512], start=(mt == 1),
                                    stop=False)
                                nc.tensor.matmul(
                                    ops[j][:, 512:1024], fva[:, h, mt - 1, :],
                                    ets[j][:, 512:1024], start=(mt == 1),
                                    stop=False)
                        ets = net
                    for j in range(2):
                        h = 2 * i + j
                        nc.tensor.matmul(ops[j][:, 0:512],
                                         fva[:, h, NT - 1, :],
                                         ets[j][:, 0:512], start=False,
                                         stop=True)
                        nc.tensor.matmul(ops[j][:, 512:1024],
                                         fva[:, h, NT - 1, :],
                                         ets[j][:, 512:1024], start=False,
                                         stop=True)
                    # epilogue: evict PSUM fast, then normalize from SBUF
                    otr = [None, None]
                    for j in range(2):
                        o = sp2.tile([D + 1, N], F32, tag="otr")
                        nc.vector.tensor_copy(o[:], ops[j][:])
                        otr[j] = o
                    for j in range(2):
                        h = 2 * i + j
                        nc.sync.dma_start(
                            den[h * 16:(h + 1) * 16, :],
                            otr[j][D:D + 1, :].rearrange(
                                "o (c f) -> o c f", f=D))
                    nc.vector.reciprocal(rec[2 * i * 16:(2 * i + 2) * 16, :],
                                         den[2 * i * 16:(2 * i + 2) * 16, :])
                    for j in range(2):
                        h = 2 * i + j
                        nc.sync.dma_start(
                            rw8[0:1, h, :].rearrange("o (c f) -> o c f", f=D),
                            rec[h * 16:(h + 1) * 16, :])
                        rw = sp2.tile([64, N], F32, tag="rw")
                        nc.gpsimd.partition_broadcast(rw[:], rw8[0:1, h, :],
                                                      channels=64)
                        if j == 0:
                            nc.vector.tensor_tensor(OT2[0:64, i, :],
                                                    otr[j][0:D, :], rw[:],
                                                    ALU.mult)
                        else:
                            ott = sp2.tile([64, N], BF16, tag="ott")
                            nc.vector.tensor_tensor(ott[:], otr[j][0:D, :],
                                                    rw[:], ALU.mult)
                            nc.sync.dma_start(OT2[64:128, i, :], ott[:])

            with (
                tc.tile_pool(name="p3", bufs=3) as sp3,
                tc.tile_pool(name="psF", bufs=2, space="PSUM") as psF,
            ):
                for nt in range(NT):
                    fps = psF.tile([128, DIM], F32, tag="fps")
                    for i in range(4):
                        nc.tensor.matmul(fps[:],
                                         OT2[:, i, nt * 128:(nt + 1) * 128],
                                         Wo_sb[:, i, :], start=(i == 0),
                                         stop=(i == 3))
                    obt = sp3.tile([128, DIM], F32, tag="obt")
                    nc.vector.tensor_tensor(obt[:], fps[:], bo_bc[:], ALU.add)
                    nc.sync.dma_start(out_e[nt * 128:(nt + 1) * 128, :],
                                      obt[:])

    nc.finalize()
    return nc


_NC1 = None
_NC2 = None


def _get_ncs():
    global _NC1, _NC2
    if _NC1 is None:
        _NC1 = build_nc1()
        _NC2 = build_nc2()
    return _NC1, _NC2


def host_mid(stats_list, qg_list, wp_W1, wp_b1, wp_ln_g, wp_ln_b, wp_W2,
             wp_b2, wp_W3, wp_b3, weight_temp):
    """Reduce per-core stats, run the weight-predictor MLP + global stds,
    return the [128, H] pass-2 scale tile (a2 rows 0:64, b2 rows 64:128)."""
    f8 = np.float64
    # --- global feature means ---
    qg = np.zeros((2, DIM), f8)
    for g in qg_list:
        qg += np.asarray(g, f8)
    q_g = qg[0].reshape(H, D) / (NCORES * N)
    k_g = qg[1].reshape(H, D) / (NCORES * N)

    # --- per-head moment sums (same conventions the baseline validated) ---
    S2C = np.zeros(H, f8)
    S2V = np.zeros(H, f8)
    SCV = np.zeros(H, f8)
    S1C = np.zeros(H, f8)
    S1VAR = np.zeros(H, f8)
    S2VAR = np.zeros(H, f8)
    SCVAR = np.zeros(H, f8)
    for s in stats_list:
        s = np.asarray(s, f8)
        S2C += s[0:64, 0:8].sum(axis=0)
        S2V += s[0:64, 8:16].sum(axis=0)
        SCV += s[0:64, 16:24].sum(axis=0)
        sqv = s[0:64, 24:32]
        skv = s[0:64, 32:40]
        S1C += (sqv * skv).sum(axis=0)
        # m1: per-(hg, ms) relu accum columns, summed over key partitions
        m1 = s[:, 40:72].sum(axis=0).reshape(H, 4).sum(axis=1)
        m2 = s[0, 72:88].reshape(H, 2).sum(axis=1)
        m3 = s[0, 88:104].reshape(H, 2).sum(axis=1)
        S1VAR += m1 / MS
        S2VAR += m2 / (MS * MS)
        SCVAR += m3 / MS
    S2V *= COV_SCALE * COV_SCALE
    SCV *= COV_SCALE

    # --- weight predictor MLP ---
    z = np.concatenate([q_g, k_g], axis=-1) @ np.asarray(wp_W1, f8) \
        + np.asarray(wp_b1, f8)
    mu = z.mean(-1, keepdims=True)
    var = z.var(-1, keepdims=True)
    z = (z - mu) / np.sqrt(var + 1e-5) * np.asarray(wp_ln_g, f8) \
        + np.asarray(wp_ln_b, f8)
    z = np.maximum(z, 0)
    z = np.maximum(z @ np.asarray(wp_W2, f8) + np.asarray(wp_b2, f8), 0)
    logits = z @ np.asarray(wp_W3, f8) + np.asarray(wp_b3, f8)
    e = np.exp(logits - logits.max(-1, keepdims=True))
    p = e / e.sum(-1, keepdims=True)
    wt = np.clip(np.asarray(weight_temp, f8), 0.1, 10.0)
    e2 = np.exp(p / wt - (p / wt).max(-1, keepdims=True))
    w = e2 / e2.sum(-1, keepdims=True)
    w = w * 0.7 + 0.1
    cw, covw, vw = w[:, 0], w[:, 1], w[:, 2]

    def std1(s1, s2):
        return np.sqrt(max((s2 - s1 * s1 / M_TOT) / (M_TOT - 1.0), 0.0))

    cos_n = std1(S1C.sum(), S2C.sum()) + EPS
    cov_n = std1(0.0, S2V.sum()) + EPS
    var_n = std1(S1VAR.sum(), S2VAR.sum()) + EPS
    A = cw / cos_n
    Bc = covw / cov_n * 0.3
    C = vw / var_n * 0.3
    S1d = (A * S1C + C * S1VAR).sum()
    S2d = (A * A * S2C + Bc * Bc * S2V + C * C * S2VAR + 2 * A * Bc * SCV
           + 2 * A * C * SCVAR).sum()
    temp = np.clip(0.5 + std1(S1d, S2d), 0.3, 3.0)
    a2 = A / temp
    b2 = Bc / temp * COV_SCALE
    sc = np.zeros((128, H), np.float32)
    sc[0:64, :] = a2[None, :]
    sc[64:128, :] = b2[None, :]
    return sc


def make_in_maps1(q, k, v, ln_g, ln_b, W_in):
    import ml_dtypes
    f = np.float32
    bf = ml_dtypes.bfloat16
    Wf = (np.asarray(ln_g, f)[:, None] * np.asarray(W_in, f)).astype(bf)
    bW = (np.asarray(ln_b, f) @ np.asarray(W_in, f))[None, :].astype(f)
    shared = dict(Wf=Wf, bW=bW)
    maps = []
    for b in range(NCORES):
        m = dict(shared)
        m["q"] = np.ascontiguousarray(np.asarray(q, f)[b])
        m["k"] = np.ascontiguousarray(np.asarray(k, f)[b])
        m["v"] = np.ascontiguousarray(np.asarray(v, f)[b])
        maps.append(m)
    return maps


def make_in_maps2(r1, sc, W_out, b_out):
    import ml_dtypes
    f = np.float32
    Wo = np.asarray(W_out, f).astype(ml_dtypes.bfloat16)
    bo = np.asarray(b_out, f)[None, :]
    maps2 = []
    for b in range(NCORES):
        maps2.append(dict(QTd=np.asarray(r1[b]["QTd"]),
                          KTd=np.asarray(r1[b]["KTd"]),
                          fvad=np.asarray(r1[b]["fvad"]),
                          sc=sc, W_out=Wo, b_out=bo))
    return maps2


def kernel(**inputs) -> np.ndarray:
    nc1, nc2 = _get_ncs()
    maps1 = make_in_maps1(inputs["q"], inputs["k"], inputs["v"],
                          inputs["ln_g"], inputs["ln_b"], inputs["W_in"])
    res1 = run_bass_kernel_spmd(nc1, maps1, core_ids=list(range(NCORES)))
    r1 = res1.results
    sc = host_mid([r1[b]["stats"] for b in range(NCORES)],
                  [r1[b]["qg"] for b in range(NCORES)],
                  inputs["wp_W1"], inputs["wp_b1"], inputs["wp_ln_g"],
                  inputs["wp_ln_b"], inputs["wp_W2"], inputs["wp_b2"],
                  inputs["wp_W3"], inputs["wp_b3"], inputs["weight_temp"])
    maps2 = make_in_maps2(r1, sc, inputs["W_out"], inputs["b_out"])
    res2 = run_bass_kernel_spmd(nc2, maps2, core_ids=list(range(NCORES)))
    r2 = res2.results
    return np.stack([np.asarray(r2[b]["out"]) for b in range(NCORES)], axis=0)


if __name__ == "__main__":
    _get_ncs()
    print("built ok")
